# revision 49
# baseline (speedup 1.0000x reference)
"""Trainium2 Bass kernel for XCiT-style channel attention.

Reference computation (per batch element b):
    qkv = x @ w_qkv + b_qkv                  # [N, 3C]
    q, k, v = split(qkv) -> per head [N, d]
    q = l2_normalize(q) * d**-0.5 ; k = l2_normalize(k)
    attn = softmax(q^T k, axis=-1)           # [h, d, d]  (contraction over tokens)
    out = v @ attn                           # [N, C]
    y = out @ w_proj + b_proj

Sharding: data-parallel over batch B=8 -> one NeuronCore per batch element.
The host feeds each core x^T (a layout choice of the sharding) and
transposes the returned y^T back; bytes moved are identical to the
natural layout.

Key algebraic trick: both l2 scales are per-(token, head) scalars in the
token contraction, so  attn = q0^T diag(s_q*s_k*scale) k0  -- only k is
scaled by s = scale * rsqrt(|q|^2) * rsqrt(|k|^2); q stays raw.

Per-core dataflow (all matmuls bf16 with fp32 PSUM accumulation):
  xT[k]   [128, N] bf16 sbuf  <- cast-DMA from xT f32 dram          (k = C chunk)
  per 128-token tile t:
    qk_ps [tok, 1024] = xT-chunks^T @ w_qkv[:, :1024]   (PE, 8 MM)
    q_bf, k_bf <- ACT cast; squares on ACT/DVE; fold+reduce -> |q|^2,|k|^2 per head
    s = 1/sqrt(max(|q|^2*|k|^2, eps^2)/scale^2)         (ACT sqrt + DVE recip)
    ks = k_bf * s (broadcast)                           (GPSIMD)
    attn_ps[:, pair] += q2^T @ ks2   2x2 head-pair packing, one PSUM bank,
                        start=False accumulate onto a memset bank (interleaved
                        chains must not use start_tensor_calc: it marks the
                        whole 2KB zero-region pending-zero)
  softmax per [64,64] head block (shift by max, ACT exp w/ accum sums)
  per token-chunk pair (512 tokens x 2):
    vT_bf[m] = w_v-chunk^T @ xT    (PE, stationary reused over chunk pair)
    outT_ps  = attn-block @ vT     (row-group packed odd/even head MMs)
    yT_ps    = w_proj-chunk^T @ outT  -> yT dram (host transposes back)
"""

import os
import sys

import numpy as np

for _p in ("/opt/trn_rl_repo", "/root/.axon_site/_ro/trn_rl_repo"):
    if _p not in sys.path and os.path.isdir(_p):
        sys.path.insert(0, _p)

os.environ.setdefault("MYCRO_LOCAL_CACHE", "1")

NUM_HEADS = 8
B, N, C = 8, 4096, 512
D = C // NUM_HEADS          # 64
SCALE = float(D) ** -0.5
EPS = 1e-12
P = 128                     # partitions / tile height
KC = C // P                 # 4 contraction chunks
NPAIR = NUM_HEADS // 2      # 4 head pairs


def _build(n_tokens=N, with_bias=False, debug_taps=False):
    """Build + compile the per-core Bass program. Returns nc."""
    import concourse.bacc as bacc
    import concourse.mybir as mybir
    import concourse.tile as tile
    from concourse.bass import ts

    f32 = mybir.dt.float32
    bf16 = mybir.dt.bfloat16
    AF = mybir.ActivationFunctionType
    AX = mybir.AxisListType

    n = n_tokens
    nt = n // P             # 128-token tiles
    tch = n // 512          # 512-token chunks
    assert tch % 2 == 0

    nc = bacc.Bacc("TRN2", target_bir_lowering=False, debug=False)

    xT = nc.dram_tensor("xT", [C, n], f32, kind="ExternalInput")
    w_qkv = nc.dram_tensor("w_qkv", [C, 3 * C], f32, kind="ExternalInput")
    w_proj = nc.dram_tensor("w_proj", [C, C], f32, kind="ExternalInput")
    if with_bias:
        b_qkv = nc.dram_tensor("b_qkv", [3 * C], f32, kind="ExternalInput")
        b_proj = nc.dram_tensor("b_proj", [C], f32, kind="ExternalInput")
    yT = nc.dram_tensor("yT", [C, n], f32, kind="ExternalOutput")

    taps = {}
    if debug_taps:
        for tname, shape in [
            ("tap_q0", [P, C]), ("tap_ks0", [P, C]), ("tap_s0", [P, NUM_HEADS]),
            ("tap_attn", [P, NPAIR * P]),
            ("tap_vT0", [P, 512]), ("tap_G0", [P, C]),
        ]:
            taps[tname] = nc.dram_tensor(tname, shape, f32, kind="ExternalOutput")

    with tile.TileContext(nc) as tc:
        with (
            tc.tile_pool(name="persist", bufs=1) as persist,
            # attn bank is allocated for the whole kernel so phase-2 psum
            # pools don't land on it and serialize behind the softmax reads
            tc.tile_pool(name="attnps", bufs=1, space="PSUM") as attn_pool,
        ):
            # ---------------- phase 0: loads (cast f32 -> bf16 in DMA) -------
            # Ordered so the first token tiles can start ASAP: the q,k halves
            # of w_qkv and the leading token-slices of xT come first; the v
            # columns and w_proj are only needed in phase 2.
            xT_sb = [persist.tile([P, n], bf16, name=f"xT{k}") for k in range(KC)]
            w_bf = [persist.tile([P, 3 * C], bf16, name=f"w_bf{k}") for k in range(KC)]
            wp_bf = [persist.tile([P, C], bf16, name=f"wp_bf{k}") for k in range(KC)]
            # token pieces, finest first so tile 0 is runnable earliest
            pieces = []
            off = 0
            while off < n:
                sz = 256 if off < 512 and n > 512 else 512
                pieces.append((off, min(sz, n - off)))
                off += sz
            for k in range(KC):
                nc.gpsimd.dma_start(out=w_bf[k][:, :C], in_=w_qkv[ts(k, P), :C])
            for pc, (off, sz) in enumerate(pieces):
                for k in range(KC):
                    nc.gpsimd.dma_start(
                        out=xT_sb[k][:, off : off + sz],
                        in_=xT[ts(k, P), off : off + sz],
                    )
                if pc == 0:
                    for k in range(KC):
                        nc.gpsimd.dma_start(
                            out=w_bf[k][:, C : 2 * C], in_=w_qkv[ts(k, P), C : 2 * C]
                        )
                if pc == min(1, len(pieces) - 1):
                    for k in range(KC):
                        nc.gpsimd.dma_start(
                            out=w_bf[k][:, 2 * C :], in_=w_qkv[ts(k, P), 2 * C :]
                        )
                    for k in range(KC):
                        nc.gpsimd.dma_start(out=wp_bf[k][:], in_=w_proj[ts(k, P), :])

            if with_bias:
                bias_qk = persist.tile([P, 2 * C], f32)
                nc.gpsimd.dma_start(
                    out=bias_qk[:], in_=b_qkv[: 2 * C].partition_broadcast(P)
                )
                bias_v = persist.tile([P, KC], f32)
                nc.gpsimd.dma_start(
                    out=bias_v[:],
                    in_=b_qkv[2 * C :].rearrange("(m p) -> p m", p=P),
                )
                bias_p = persist.tile([P, KC], f32)
                nc.gpsimd.dma_start(
                    out=bias_p[:], in_=b_proj[:].rearrange("(m p) -> p m", p=P)
                )

            # ---------------- phase 1: qk + attn accumulation ---------------
            # Four accumulation chains share one PSUM bank: memset once and
            # accumulate with start=False (see module docstring).
            if True:
                attn_ps = attn_pool.tile([P, NPAIR * P], f32)
                nc.vector.memset(attn_ps[:], 0.0)

                vT_bf = [
                    persist.tile([P, n], bf16, name=f"vT_bf{m}") for m in range(KC)
                ]
                with (
                    tc.tile_pool(name="qkvps", bufs=2, space="PSUM") as qkvps,
                    tc.tile_pool(name="vtps", bufs=2, space="PSUM") as vtps,
                    tc.tile_pool(name="work", bufs=4) as work,
                    tc.tile_pool(name="smalls", bufs=4) as smalls,
                ):
                    # software-pipelined state (tile t emits C-matmuls of t-2)
                    st = {}

                    def emit_A(t):
                        qk_ps = qkvps.tile([P, 2 * C], f32)
                        for half in range(2):
                            for k in range(KC):
                                nc.tensor.matmul(
                                    qk_ps[:, ts(half, C)],
                                    xT_sb[k][:, ts(t, P)],
                                    w_bf[k][:, ts(half, C)],
                                    start=(k == 0),
                                    stop=(k == KC - 1),
                                )
                        q_bf = work.tile([P, C], bf16)
                        sqq = work.tile([P, C], bf16)
                        sqk = work.tile([P, C], bf16)
                        if with_bias:
                            k_bf = work.tile([P, C], bf16)
                            nc.vector.tensor_add(q_bf[:], qk_ps[:, :C], bias_qk[:, :C])
                            nc.vector.tensor_add(k_bf[:], qk_ps[:, C:], bias_qk[:, C:])
                            nc.scalar.activation(sqq[:], q_bf[:], AF.Square)
                            nc.vector.tensor_mul(sqk[:], k_bf[:], k_bf[:])
                        else:
                            # k stays in PSUM: Square reads it on ACT, and the
                            # s-scaling multiply reads it again on DVE
                            k_bf = None
                            nc.scalar.copy(q_bf[:], qk_ps[:, :C])
                            nc.scalar.activation(sqk[:], qk_ps[:, C:], AF.Square)
                            nc.vector.tensor_mul(sqq[:], q_bf[:], q_bf[:])
                        # fold 64 -> 32 at 2x rate, then 1x reduce of 32
                        fq = work.tile([P, C // 2], bf16)
                        fk = work.tile([P, C // 2], bf16)
                        sqq3 = sqq[:].rearrange("p (h d) -> p h d", d=D)
                        sqk3 = sqk[:].rearrange("p (h d) -> p h d", d=D)
                        nc.vector.tensor_add(
                            fq[:].rearrange("p (h d) -> p h d", d=D // 2),
                            sqq3[:, :, 0 : D // 2],
                            sqq3[:, :, D // 2 : D],
                        )
                        nc.vector.tensor_add(
                            fk[:].rearrange("p (h d) -> p h d", d=D // 2),
                            sqk3[:, :, 0 : D // 2],
                            sqk3[:, :, D // 2 : D],
                        )
                        ssq = smalls.tile([P, NUM_HEADS], f32)
                        ssk = smalls.tile([P, NUM_HEADS], f32)
                        nc.vector.reduce_sum(
                            ssq[:], fq[:].rearrange("p (h d) -> p h d", d=D // 2), AX.X
                        )
                        nc.vector.reduce_sum(
                            ssk[:], fk[:].rearrange("p (h d) -> p h d", d=D // 2), AX.X
                        )
                        pp = smalls.tile([P, NUM_HEADS], f32)
                        nc.vector.tensor_mul(pp[:], ssq[:], ssk[:])
                        nc.vector.tensor_scalar_max(pp[:], pp[:], EPS * EPS)
                        st[t] = dict(q_bf=q_bf, k_bf=k_bf, pp=pp, qk_ps=qk_ps)

                    def emit_scale(t):
                        # sqrt on ACT (one tile late so its DVE deps are ready),
                        # then reciprocal and the broadcasted k*s on gpsimd
                        d = st[t]
                        rt = smalls.tile([P, NUM_HEADS], f32)
                        nc.scalar.activation(
                            rt[:], d["pp"][:], AF.Sqrt, scale=1.0 / (SCALE * SCALE)
                        )
                        s = smalls.tile([P, NUM_HEADS], f32)
                        nc.vector.reciprocal(s[:], rt[:])
                        ks_bf = work.tile([P, C], bf16)
                        k_src = (
                            d["k_bf"][:] if d["k_bf"] is not None else d["qk_ps"][:, C:]
                        )
                        nc.vector.tensor_mul(
                            ks_bf[:].rearrange("p (h d) -> p h d", d=D),
                            k_src.rearrange("p (h d) -> p h d", d=D),
                            s[:].to_broadcast((P, NUM_HEADS, D)),
                        )
                        d["ks_bf"] = ks_bf
                        d["s"] = s
                        if debug_taps and t == 0:
                            nc.gpsimd.dma_start(out=taps["tap_q0"][:], in_=d["q_bf"][:])
                            nc.gpsimd.dma_start(out=taps["tap_ks0"][:], in_=ks_bf[:])
                            nc.gpsimd.dma_start(out=taps["tap_s0"][:], in_=s[:])

                    def emit_C(t):
                        d = st.pop(t)
                        for pr in range(NPAIR):
                            nc.tensor.matmul(
                                attn_ps[:, ts(pr, P)],
                                d["q_bf"][:, ts(pr, P)],
                                d["ks_bf"][:, ts(pr, P)],
                                start=False,
                                stop=False,
                                skip_group_check=True,
                            )

                    # vT matmuls are independent of the attn pipeline and ride
                    # the same xT load frontier -- interleave them as PE
                    # gap-fillers, one chunk-pair group (stationary reused x2)
                    # every other tile; the DVE copy doubles as the bf16 cast.
                    def emit_B(gp):
                        m, tcp = gp % KC, gp // KC
                        vt_ps = [
                            vtps.tile([P, 512], f32, name="vt_ps") for _ in range(2)
                        ]
                        for k in range(KC):
                            for c2 in range(2):
                                nc.tensor.matmul(
                                    vt_ps[c2][:],
                                    w_bf[k][:, 2 * C + m * P : 2 * C + (m + 1) * P],
                                    xT_sb[k][:, ts(2 * tcp + c2, 512)],
                                    start=(k == 0),
                                    stop=(k == KC - 1),
                                )
                        for c2 in range(2):
                            tc_i = 2 * tcp + c2
                            if with_bias:
                                nc.scalar.add(
                                    vT_bf[m][:, ts(tc_i, 512)],
                                    vt_ps[c2][:],
                                    bias_v[:, m : m + 1],
                                )
                            else:
                                nc.vector.tensor_copy(
                                    vT_bf[m][:, ts(tc_i, 512)], vt_ps[c2][:]
                                )

                    DELAY = 2
                    BP_START = 8
                    n_bpairs = KC * tch // 2
                    next_gp = 0
                    for t in range(nt + DELAY):
                        if t < nt:
                            emit_A(t)
                        if 1 <= t <= nt:
                            emit_scale(t - 1)
                        if t >= DELAY:
                            emit_C(t - DELAY)
                        if (
                            t >= BP_START
                            and (t - BP_START) % 2 == 0
                            and next_gp < n_bpairs - 4
                        ):
                            emit_B(next_gp)
                            next_gp += 1
                    # trailing groups overlap the softmax chain on PE
                    for gp in range(next_gp, n_bpairs):
                        emit_B(gp)

                # ---------------- softmax on [64,64] head blocks -------------
                # head h: rows (h%2)*64..+64, cols (h//2)*128+(h%2)*64..+64
                # The 1/sum normalizer is NOT applied here -- it is folded into
                # the per-partition scale of the G = attn @ w_proj cast below.
                mx = persist.tile([P, NPAIR], f32)
                sums = persist.tile([P, NPAIR], f32)
                rs = persist.tile([P, NPAIR], f32)
                attn3 = attn_ps[:].rearrange("r (pr c) -> r pr c", c=P)
                nc.vector.reduce_max(mx[0:D, :], attn3[0:D, :, 0:D], AX.X, negate=True)
                nc.vector.reduce_max(mx[D:P, :], attn3[D:P, :, D:P], AX.X, negate=True)
                attn_eb = persist.tile([P, NPAIR * P], bf16)
                if debug_taps:
                    nc.vector.memset(attn_eb[:], 0.0)
                for h in range(NUM_HEADS):
                    pr, odd = h // 2, h % 2
                    r0 = odd * D
                    c0 = pr * P + odd * D
                    nc.scalar.activation(
                        attn_eb[r0 : r0 + D, c0 : c0 + D],
                        attn_ps[r0 : r0 + D, c0 : c0 + D],
                        AF.Exp,
                        bias=mx[r0 : r0 + D, pr : pr + 1],
                        accum_out=sums[r0 : r0 + D, pr : pr + 1],
                    )
                nc.vector.reciprocal(rs[:], sums[:])
                if debug_taps:
                    nc.gpsimd.dma_start(out=taps["tap_attn"][:], in_=attn_eb[:])

            # ------------- phase 2: G = softmax(attn) @ w_proj, then yT ------
            with (
                tc.tile_pool(name="tps", bufs=1, space="PSUM") as tpool,
                tc.tile_pool(name="gps", bufs=2, space="PSUM") as gps,
                tc.tile_pool(name="ytps", bufs=4, space="PSUM") as ytps,
                tc.tile_pool(name="ytsb", bufs=4) as ytsb,
            ):
                # G[(h,d), c'] = sum_e softmax-unnormalized attn_h[d,e] *
                # w_proj[h*64+e, c'], scaled by rs[(h,d)] at the PSUM cast.
                # Then yT = G^T-contraction over (h,d) with vT -- the outT
                # stage disappears entirely (y = v @ (attn @ w_proj)).
                ident = persist.tile([P, P], bf16)
                from concourse.masks import make_identity

                make_identity(nc, ident[:])
                attnT_bf = persist.tile([P, NPAIR * P], bf16)
                nc.vector.memset(attnT_bf[:], 0.0)
                G_bf = [persist.tile([P, C], bf16, name=f"G_bf{j}") for j in range(KC)]

                def emit_G():
                    tps_t = tpool.tile([P, NPAIR * P], bf16)
                    nc.vector.memset(tps_t[:].bitcast(mybir.dt.float32), 0.0)
                    for h in range(NUM_HEADS):
                        pr, odd = h // 2, h % 2
                        r0 = odd * D
                        c0 = pr * P + odd * D
                        nc.tensor.transpose(
                            tps_t[r0 : r0 + D, c0 : c0 + D],
                            attn_eb[r0 : r0 + D, c0 : c0 + D],
                            ident[r0 : r0 + D, r0 : r0 + D],
                        )
                    for j in range(KC):
                        nc.scalar.copy(attnT_bf[:, ts(j, P)], tps_t[:, ts(j, P)])
                    for j in range(KC):
                        g_ps = gps.tile([P, C], f32)
                        for odd in range(2):
                            r0 = odd * D
                            nc.tensor.matmul(
                                g_ps[r0 : r0 + D, :],
                                attnT_bf[r0 : r0 + D, j * P + r0 : j * P + r0 + D],
                                wp_bf[j][r0 : r0 + D, :],
                            )
                        # fold the softmax 1/sum (per output row (h,d)) here
                        nc.scalar.mul(G_bf[j][:], g_ps[:], rs[:, j : j + 1])

                emit_G()
                if debug_taps:
                    nc.gpsimd.dma_start(out=taps["tap_vT0"][:], in_=vT_bf[0][:, 0:512])
                    nc.gpsimd.dma_start(out=taps["tap_G0"][:], in_=G_bf[0][:])

                # yT = G^T-contraction with vT; stationary G reused x4 chunks
                ngrp = min(4, tch)
                for tcq in range(tch // ngrp):
                    for mp in range(KC):
                        yt_ps = [
                            ytps.tile([P, 512], f32, name="yt_ps") for _ in range(ngrp)
                        ]
                        for j in range(KC):
                            for cg in range(ngrp):
                                nc.tensor.matmul(
                                    yt_ps[cg][:],
                                    G_bf[j][:, ts(mp, P)],
                                    vT_bf[j][:, ts(ngrp * tcq + cg, 512)],
                                    start=(j == 0),
                                    stop=(j == KC - 1),
                                )
                        for cg in range(ngrp):
                            tc_i = ngrp * tcq + cg
                            yt_sb = ytsb.tile([P, 512], f32)
                            if with_bias:
                                nc.scalar.add(
                                    yt_sb[:], yt_ps[cg][:], bias_p[:, mp : mp + 1]
                                )
                            elif cg % 2 == 0:
                                nc.vector.tensor_copy(yt_sb[:], yt_ps[cg][:])
                            else:
                                nc.scalar.copy(yt_sb[:], yt_ps[cg][:])
                            nc.sync.dma_start(
                                yT[ts(mp, P), ts(tc_i, 512)], yt_sb[:]
                            )

    nc.compile()
    return nc


_CACHE = {}


def _get_nc(n_tokens=N, with_bias=False):
    key = (n_tokens, with_bias)
    if key not in _CACHE:
        _CACHE[key] = _build(n_tokens, with_bias)
    return _CACHE[key]


def run(inputs, trace=False, n_tokens=N):
    """Run on 8 NeuronCores; returns (y [B,N,C] f32, BassKernelResults)."""
    from concourse import bass_utils

    x = np.asarray(inputs["x"], dtype=np.float32)
    w_qkv = np.asarray(inputs["w_qkv"], dtype=np.float32)
    b_qkv = np.asarray(inputs["b_qkv"], dtype=np.float32)
    w_proj = np.asarray(inputs["w_proj"], dtype=np.float32)
    b_proj = np.asarray(inputs["b_proj"], dtype=np.float32)

    with_bias = bool(np.any(b_qkv) or np.any(b_proj))
    nc = _get_nc(n_tokens, with_bias)

    in_maps = []
    for b in range(B):
        m = {
            "xT": np.ascontiguousarray(x[b].T),
            "w_qkv": w_qkv,
            "w_proj": w_proj,
        }
        if with_bias:
            m["b_qkv"] = b_qkv
            m["b_proj"] = b_proj
        in_maps.append(m)

    res = bass_utils.run_bass_kernel_spmd(
        nc, in_maps, core_ids=list(range(B)), trace=trace
    )
    out = np.stack(
        [np.ascontiguousarray(res.results[i]["yT"].T) for i in range(B)], axis=0
    )
    return out.astype(np.float32), res


def kernel(**inputs):
    out, _ = run(inputs)
    return out


# revision 51
# speedup vs baseline: 1.1835x; 1.1835x over previous
"""Trainium2 Bass kernel for XCiT-style channel attention.

Reference computation (per batch element b):
    qkv = x @ w_qkv + b_qkv                  # [N, 3C]
    q, k, v = split(qkv) -> per head [N, d]
    q = l2_normalize(q) * d**-0.5 ; k = l2_normalize(k)
    attn = softmax(q^T k, axis=-1)           # [h, d, d]  (contraction over tokens)
    out = v @ attn                           # [N, C]
    y = out @ w_proj + b_proj

Sharding: data-parallel over batch B=8 -> one NeuronCore per batch element.
The host feeds each core x^T (a layout choice of the sharding) and
transposes the returned y^T back; bytes moved are identical to the
natural layout.

Key algebraic trick: both l2 scales are per-(token, head) scalars in the
token contraction, so  attn = q0^T diag(s_q*s_k*scale) k0  -- only k is
scaled by s = scale * rsqrt(|q|^2) * rsqrt(|k|^2); q stays raw.

Per-core dataflow (all matmuls bf16 with fp32 PSUM accumulation):
  xT[k]   [128, N] bf16 sbuf  <- cast-DMA from xT f32 dram          (k = C chunk)
  per 128-token tile t:
    qk_ps [tok, 1024] = xT-chunks^T @ w_qkv[:, :1024]   (PE, 8 MM)
    q_bf, k_bf <- ACT cast; squares on ACT/DVE; fold+reduce -> |q|^2,|k|^2 per head
    s = 1/sqrt(max(|q|^2*|k|^2, eps^2)/scale^2)         (ACT sqrt + DVE recip)
    ks = k_bf * s (broadcast)                           (GPSIMD)
    attn_ps[:, pair] += q2^T @ ks2   2x2 head-pair packing, one PSUM bank,
                        start=False accumulate onto a memset bank (interleaved
                        chains must not use start_tensor_calc: it marks the
                        whole 2KB zero-region pending-zero)
  softmax per [64,64] head block (shift by max, ACT exp w/ accum sums)
  per token-chunk pair (512 tokens x 2):
    vT_bf[m] = w_v-chunk^T @ xT    (PE, stationary reused over chunk pair)
    outT_ps  = attn-block @ vT     (row-group packed odd/even head MMs)
    yT_ps    = w_proj-chunk^T @ outT  -> yT dram (host transposes back)
"""

import os
import sys

import numpy as np

for _p in ("/opt/trn_rl_repo", "/root/.axon_site/_ro/trn_rl_repo"):
    if _p not in sys.path and os.path.isdir(_p):
        sys.path.insert(0, _p)

os.environ.setdefault("MYCRO_LOCAL_CACHE", "1")

NUM_HEADS = 8
B, N, C = 8, 4096, 512
D = C // NUM_HEADS          # 64
SCALE = float(D) ** -0.5
EPS = 1e-12
P = 128                     # partitions / tile height
KC = C // P                 # 4 contraction chunks
NPAIR = NUM_HEADS // 2      # 4 head pairs


def _build(n_tokens=N, with_bias=False, debug_taps=False):
    """Build + compile the per-core Bass program. Returns nc."""
    import concourse.bacc as bacc
    import concourse.mybir as mybir
    import concourse.tile as tile
    from concourse.bass import ts

    f32 = mybir.dt.float32
    bf16 = mybir.dt.bfloat16
    AF = mybir.ActivationFunctionType
    AX = mybir.AxisListType

    n = n_tokens
    nt = n // P             # 128-token tiles
    tch = n // 512          # 512-token chunks
    assert tch % 2 == 0

    nc = bacc.Bacc("TRN2", target_bir_lowering=False, debug=False)

    xT = nc.dram_tensor("xT", [C, n], f32, kind="ExternalInput")
    w_qkv = nc.dram_tensor("w_qkv", [C, 3 * C], f32, kind="ExternalInput")
    w_proj = nc.dram_tensor("w_proj", [C, C], f32, kind="ExternalInput")
    if with_bias:
        b_qkv = nc.dram_tensor("b_qkv", [3 * C], f32, kind="ExternalInput")
        b_proj = nc.dram_tensor("b_proj", [C], f32, kind="ExternalInput")
    yT = nc.dram_tensor("yT", [C, n], f32, kind="ExternalOutput")

    taps = {}
    if debug_taps:
        for tname, shape in [
            ("tap_q0", [P, C]), ("tap_ks0", [P, C]), ("tap_s0", [P, NUM_HEADS]),
            ("tap_attn", [P, NPAIR * P]),
            ("tap_vT0", [P, 512]), ("tap_G0", [P, C]),
        ]:
            taps[tname] = nc.dram_tensor(tname, shape, f32, kind="ExternalOutput")

    with tile.TileContext(nc) as tc:
        with (
            tc.tile_pool(name="persist", bufs=1) as persist,
            # attn bank is allocated for the whole kernel so phase-2 psum
            # pools don't land on it and serialize behind the softmax reads
            tc.tile_pool(name="attnps", bufs=1, space="PSUM") as attn_pool,
        ):
            # ---------------- phase 0: loads (cast f32 -> bf16 in DMA) -------
            # Ordered so the first token tiles can start ASAP: the q,k halves
            # of w_qkv and the leading token-slices of xT come first; the v
            # columns and w_proj are only needed in phase 2.
            xT_sb = [persist.tile([P, n], bf16, name=f"xT{k}") for k in range(KC)]
            w_bf = [persist.tile([P, 3 * C], bf16, name=f"w_bf{k}") for k in range(KC)]
            wp_bf = [persist.tile([P, C], bf16, name=f"wp_bf{k}") for k in range(KC)]
            # token pieces, finest first so tile 0 is runnable earliest
            pieces = []
            off = 0
            while off < n:
                sz = 256 if off < 512 and n > 512 else 512
                pieces.append((off, min(sz, n - off)))
                off += sz
            for k in range(KC):
                nc.gpsimd.dma_start(out=w_bf[k][:, :C], in_=w_qkv[ts(k, P), :C])
            for pc, (off, sz) in enumerate(pieces):
                for k in range(KC):
                    nc.gpsimd.dma_start(
                        out=xT_sb[k][:, off : off + sz],
                        in_=xT[ts(k, P), off : off + sz],
                    )
                if pc == 0:
                    for k in range(KC):
                        nc.gpsimd.dma_start(
                            out=w_bf[k][:, C : 2 * C], in_=w_qkv[ts(k, P), C : 2 * C]
                        )
                if pc == min(1, len(pieces) - 1):
                    for k in range(KC):
                        nc.gpsimd.dma_start(
                            out=w_bf[k][:, 2 * C :], in_=w_qkv[ts(k, P), 2 * C :]
                        )
                    for k in range(KC):
                        nc.gpsimd.dma_start(out=wp_bf[k][:], in_=w_proj[ts(k, P), :])

            if with_bias:
                bias_qk = persist.tile([P, 2 * C], f32)
                nc.gpsimd.dma_start(
                    out=bias_qk[:], in_=b_qkv[: 2 * C].partition_broadcast(P)
                )
                bias_v = persist.tile([P, KC], f32)
                nc.gpsimd.dma_start(
                    out=bias_v[:],
                    in_=b_qkv[2 * C :].rearrange("(m p) -> p m", p=P),
                )
                bias_p = persist.tile([P, KC], f32)
                nc.gpsimd.dma_start(
                    out=bias_p[:], in_=b_proj[:].rearrange("(m p) -> p m", p=P)
                )

            # ---------------- phase 1: qk + attn accumulation ---------------
            # Four accumulation chains share one PSUM bank: memset once and
            # accumulate with start=False (see module docstring).
            if True:
                attn_ps = attn_pool.tile([P, NPAIR * P], f32)
                nc.vector.memset(attn_ps[:], 0.0)

                vT_bf = [
                    persist.tile([P, n], bf16, name=f"vT_bf{m}") for m in range(KC)
                ]
                with (
                    tc.tile_pool(name="qkvps", bufs=2, space="PSUM") as qkvps,
                    tc.tile_pool(name="vtps", bufs=2, space="PSUM") as vtps,
                    tc.tile_pool(name="work", bufs=4) as work,
                    tc.tile_pool(name="smalls", bufs=4) as smalls,
                ):
                    # software-pipelined state (tile t emits C-matmuls of t-2)
                    st = {}

                    def emit_A(t):
                        qk_ps = qkvps.tile([P, 2 * C], f32)
                        for half in range(2):
                            for k in range(KC):
                                nc.tensor.matmul(
                                    qk_ps[:, ts(half, C)],
                                    xT_sb[k][:, ts(t, P)],
                                    w_bf[k][:, ts(half, C)],
                                    start=(k == 0),
                                    stop=(k == KC - 1),
                                )
                        q_bf = work.tile([P, C], bf16)
                        k_bf = work.tile([P, C], bf16)
                        sqq = work.tile([P, C], bf16)
                        sqk = work.tile([P, C], bf16)
                        if with_bias:
                            nc.vector.tensor_add(q_bf[:], qk_ps[:, :C], bias_qk[:, :C])
                            nc.vector.tensor_add(k_bf[:], qk_ps[:, C:], bias_qk[:, C:])
                            nc.scalar.activation(sqq[:], q_bf[:], AF.Square)
                            nc.vector.tensor_mul(sqk[:], k_bf[:], k_bf[:])
                        else:
                            nc.scalar.copy(q_bf[:], qk_ps[:, :C])
                            nc.scalar.copy(k_bf[:], qk_ps[:, C:])
                            nc.scalar.activation(sqq[:], q_bf[:], AF.Square)
                            nc.vector.tensor_mul(sqk[:], k_bf[:], k_bf[:])
                        # fold 64 -> 32 at 2x rate, then 1x reduce of 32
                        fq = work.tile([P, C // 2], bf16)
                        fk = work.tile([P, C // 2], bf16)
                        sqq3 = sqq[:].rearrange("p (h d) -> p h d", d=D)
                        sqk3 = sqk[:].rearrange("p (h d) -> p h d", d=D)
                        nc.vector.tensor_add(
                            fq[:].rearrange("p (h d) -> p h d", d=D // 2),
                            sqq3[:, :, 0 : D // 2],
                            sqq3[:, :, D // 2 : D],
                        )
                        nc.vector.tensor_add(
                            fk[:].rearrange("p (h d) -> p h d", d=D // 2),
                            sqk3[:, :, 0 : D // 2],
                            sqk3[:, :, D // 2 : D],
                        )
                        ssq = smalls.tile([P, NUM_HEADS], f32)
                        ssk = smalls.tile([P, NUM_HEADS], f32)
                        nc.vector.reduce_sum(
                            ssq[:], fq[:].rearrange("p (h d) -> p h d", d=D // 2), AX.X
                        )
                        nc.vector.reduce_sum(
                            ssk[:], fk[:].rearrange("p (h d) -> p h d", d=D // 2), AX.X
                        )
                        pp = smalls.tile([P, NUM_HEADS], f32)
                        nc.vector.tensor_mul(pp[:], ssq[:], ssk[:])
                        nc.vector.tensor_scalar_max(pp[:], pp[:], EPS * EPS)
                        st[t] = dict(q_bf=q_bf, k_bf=k_bf, pp=pp, qk_ps=qk_ps)

                    def emit_scale(t):
                        # sqrt on ACT (one tile late so its DVE deps are ready),
                        # then reciprocal and the broadcasted k*s on gpsimd
                        d = st[t]
                        rt = smalls.tile([P, NUM_HEADS], f32)
                        nc.scalar.activation(
                            rt[:], d["pp"][:], AF.Sqrt, scale=1.0 / (SCALE * SCALE)
                        )
                        s = smalls.tile([P, NUM_HEADS], f32)
                        nc.vector.reciprocal(s[:], rt[:])
                        ks_bf = work.tile([P, C], bf16)
                        nc.vector.tensor_mul(
                            ks_bf[:].rearrange("p (h d) -> p h d", d=D),
                            d["k_bf"][:].rearrange("p (h d) -> p h d", d=D),
                            s[:].to_broadcast((P, NUM_HEADS, D)),
                        )
                        d["ks_bf"] = ks_bf
                        d["s"] = s
                        if debug_taps and t == 0:
                            nc.gpsimd.dma_start(out=taps["tap_q0"][:], in_=d["q_bf"][:])
                            nc.gpsimd.dma_start(out=taps["tap_ks0"][:], in_=ks_bf[:])
                            nc.gpsimd.dma_start(out=taps["tap_s0"][:], in_=s[:])

                    def emit_C(t):
                        d = st.pop(t)
                        for pr in range(NPAIR):
                            nc.tensor.matmul(
                                attn_ps[:, ts(pr, P)],
                                d["q_bf"][:, ts(pr, P)],
                                d["ks_bf"][:, ts(pr, P)],
                                start=False,
                                stop=False,
                                skip_group_check=True,
                            )

                    # vT matmuls are independent of the attn pipeline and ride
                    # the same xT load frontier -- interleave them as PE
                    # gap-fillers, one chunk-pair group (stationary reused x2)
                    # every other tile; the DVE copy doubles as the bf16 cast.
                    def emit_B(gp):
                        m, tcp = gp % KC, gp // KC
                        vt_ps = [
                            vtps.tile([P, 512], f32, name="vt_ps") for _ in range(2)
                        ]
                        for k in range(KC):
                            for c2 in range(2):
                                nc.tensor.matmul(
                                    vt_ps[c2][:],
                                    w_bf[k][:, 2 * C + m * P : 2 * C + (m + 1) * P],
                                    xT_sb[k][:, ts(2 * tcp + c2, 512)],
                                    start=(k == 0),
                                    stop=(k == KC - 1),
                                )
                        for c2 in range(2):
                            tc_i = 2 * tcp + c2
                            if with_bias:
                                nc.scalar.add(
                                    vT_bf[m][:, ts(tc_i, 512)],
                                    vt_ps[c2][:],
                                    bias_v[:, m : m + 1],
                                )
                            else:
                                nc.vector.tensor_copy(
                                    vT_bf[m][:, ts(tc_i, 512)], vt_ps[c2][:]
                                )

                    DELAY = 2
                    BP_START = 8
                    n_bpairs = KC * tch // 2
                    next_gp = 0
                    for t in range(nt + DELAY):
                        if t < nt:
                            emit_A(t)
                        if 1 <= t <= nt:
                            emit_scale(t - 1)
                        if t >= DELAY:
                            emit_C(t - DELAY)
                        if (
                            t >= BP_START
                            and (t - BP_START) % 2 == 0
                            and next_gp < n_bpairs - 4
                        ):
                            emit_B(next_gp)
                            next_gp += 1
                    # trailing groups overlap the softmax chain on PE
                    for gp in range(next_gp, n_bpairs):
                        emit_B(gp)

                # ---------------- softmax on [64,64] head blocks -------------
                # head h: rows (h%2)*64..+64, cols (h//2)*128+(h%2)*64..+64
                # The 1/sum normalizer is NOT applied here -- it is folded into
                # the per-partition scale of the G = attn @ w_proj cast below.
                mx = persist.tile([P, NPAIR], f32)
                sums = persist.tile([P, NPAIR], f32)
                rs = persist.tile([P, NPAIR], f32)
                attn3 = attn_ps[:].rearrange("r (pr c) -> r pr c", c=P)
                nc.vector.reduce_max(mx[0:D, :], attn3[0:D, :, 0:D], AX.X, negate=True)
                nc.vector.reduce_max(mx[D:P, :], attn3[D:P, :, D:P], AX.X, negate=True)
                attn_eb = persist.tile([P, NPAIR * P], bf16)
                if debug_taps:
                    nc.vector.memset(attn_eb[:], 0.0)
                for h in range(NUM_HEADS):
                    pr, odd = h // 2, h % 2
                    r0 = odd * D
                    c0 = pr * P + odd * D
                    nc.scalar.activation(
                        attn_eb[r0 : r0 + D, c0 : c0 + D],
                        attn_ps[r0 : r0 + D, c0 : c0 + D],
                        AF.Exp,
                        bias=mx[r0 : r0 + D, pr : pr + 1],
                        accum_out=sums[r0 : r0 + D, pr : pr + 1],
                    )
                nc.vector.reciprocal(rs[:], sums[:])
                if debug_taps:
                    nc.gpsimd.dma_start(out=taps["tap_attn"][:], in_=attn_eb[:])

            # ------------- phase 2: G = softmax(attn) @ w_proj, then yT ------
            with (
                tc.tile_pool(name="tps", bufs=1, space="PSUM") as tpool,
                tc.tile_pool(name="gps", bufs=2, space="PSUM") as gps,
                tc.tile_pool(name="ytps", bufs=4, space="PSUM") as ytps,
                tc.tile_pool(name="ytsb", bufs=4) as ytsb,
            ):
                # G[(h,d), c'] = sum_e softmax-unnormalized attn_h[d,e] *
                # w_proj[h*64+e, c'], scaled by rs[(h,d)] at the PSUM cast.
                # Then yT = G^T-contraction over (h,d) with vT -- the outT
                # stage disappears entirely (y = v @ (attn @ w_proj)).
                ident = persist.tile([P, P], bf16)
                from concourse.masks import make_identity

                make_identity(nc, ident[:])
                attnT_bf = persist.tile([P, NPAIR * P], bf16)
                nc.vector.memset(attnT_bf[:], 0.0)
                G_bf = [persist.tile([P, C], bf16, name=f"G_bf{j}") for j in range(KC)]

                def emit_G():
                    tps_t = tpool.tile([P, NPAIR * P], bf16)
                    nc.vector.memset(tps_t[:].bitcast(mybir.dt.float32), 0.0)
                    for h in range(NUM_HEADS):
                        pr, odd = h // 2, h % 2
                        r0 = odd * D
                        c0 = pr * P + odd * D
                        nc.tensor.transpose(
                            tps_t[r0 : r0 + D, c0 : c0 + D],
                            attn_eb[r0 : r0 + D, c0 : c0 + D],
                            ident[r0 : r0 + D, r0 : r0 + D],
                        )
                    for j in range(KC):
                        nc.scalar.copy(attnT_bf[:, ts(j, P)], tps_t[:, ts(j, P)])
                    for j in range(KC):
                        g_ps = gps.tile([P, C], f32)
                        for odd in range(2):
                            r0 = odd * D
                            nc.tensor.matmul(
                                g_ps[r0 : r0 + D, :],
                                attnT_bf[r0 : r0 + D, j * P + r0 : j * P + r0 + D],
                                wp_bf[j][r0 : r0 + D, :],
                            )
                        # fold the softmax 1/sum (per output row (h,d)) here
                        nc.scalar.mul(G_bf[j][:], g_ps[:], rs[:, j : j + 1])

                emit_G()
                if debug_taps:
                    nc.gpsimd.dma_start(out=taps["tap_vT0"][:], in_=vT_bf[0][:, 0:512])
                    nc.gpsimd.dma_start(out=taps["tap_G0"][:], in_=G_bf[0][:])

                # yT = G^T-contraction with vT; stationary G reused x4 chunks
                ngrp = min(4, tch)
                for tcq in range(tch // ngrp):
                    for mp in range(KC):
                        yt_ps = [
                            ytps.tile([P, 512], f32, name="yt_ps") for _ in range(ngrp)
                        ]
                        for j in range(KC):
                            for cg in range(ngrp):
                                nc.tensor.matmul(
                                    yt_ps[cg][:],
                                    G_bf[j][:, ts(mp, P)],
                                    vT_bf[j][:, ts(ngrp * tcq + cg, 512)],
                                    start=(j == 0),
                                    stop=(j == KC - 1),
                                )
                        for cg in range(ngrp):
                            tc_i = ngrp * tcq + cg
                            yt_sb = ytsb.tile([P, 512], f32)
                            if with_bias:
                                nc.scalar.add(
                                    yt_sb[:], yt_ps[cg][:], bias_p[:, mp : mp + 1]
                                )
                            elif cg % 2 == 0:
                                nc.vector.tensor_copy(yt_sb[:], yt_ps[cg][:])
                            else:
                                nc.scalar.copy(yt_sb[:], yt_ps[cg][:])
                            nc.sync.dma_start(
                                yT[ts(mp, P), ts(tc_i, 512)], yt_sb[:]
                            )

    nc.compile()
    return nc


_CACHE = {}


def _get_nc(n_tokens=N, with_bias=False):
    key = (n_tokens, with_bias)
    if key not in _CACHE:
        _CACHE[key] = _build(n_tokens, with_bias)
    return _CACHE[key]


def run(inputs, trace=False, n_tokens=N):
    """Run on 8 NeuronCores; returns (y [B,N,C] f32, BassKernelResults)."""
    from concourse import bass_utils

    x = np.asarray(inputs["x"], dtype=np.float32)
    w_qkv = np.asarray(inputs["w_qkv"], dtype=np.float32)
    b_qkv = np.asarray(inputs["b_qkv"], dtype=np.float32)
    w_proj = np.asarray(inputs["w_proj"], dtype=np.float32)
    b_proj = np.asarray(inputs["b_proj"], dtype=np.float32)

    with_bias = bool(np.any(b_qkv) or np.any(b_proj))
    nc = _get_nc(n_tokens, with_bias)

    in_maps = []
    for b in range(B):
        m = {
            "xT": np.ascontiguousarray(x[b].T),
            "w_qkv": w_qkv,
            "w_proj": w_proj,
        }
        if with_bias:
            m["b_qkv"] = b_qkv
            m["b_proj"] = b_proj
        in_maps.append(m)

    res = bass_utils.run_bass_kernel_spmd(
        nc, in_maps, core_ids=list(range(B)), trace=trace
    )
    out = np.stack(
        [np.ascontiguousarray(res.results[i]["yT"].T) for i in range(B)], axis=0
    )
    return out.astype(np.float32), res


def kernel(**inputs):
    out, _ = run(inputs)
    return out


# revision 53
# speedup vs baseline: 1.2251x; 1.0351x over previous
"""Trainium2 Bass kernel for XCiT-style channel attention.

Reference computation (per batch element b):
    qkv = x @ w_qkv + b_qkv                  # [N, 3C]
    q, k, v = split(qkv) -> per head [N, d]
    q = l2_normalize(q) * d**-0.5 ; k = l2_normalize(k)
    attn = softmax(q^T k, axis=-1)           # [h, d, d]  (contraction over tokens)
    out = v @ attn                           # [N, C]
    y = out @ w_proj + b_proj

Sharding: data-parallel over batch B=8 -> one NeuronCore per batch element.
The host feeds each core x^T (a layout choice of the sharding) and
transposes the returned y^T back; bytes moved are identical to the
natural layout.

Key algebraic trick: both l2 scales are per-(token, head) scalars in the
token contraction, so  attn = q0^T diag(s_q*s_k*scale) k0  -- only k is
scaled by s = scale * rsqrt(|q|^2) * rsqrt(|k|^2); q stays raw.

Per-core dataflow (all matmuls bf16 with fp32 PSUM accumulation):
  xT[k]   [128, N] bf16 sbuf  <- cast-DMA from xT f32 dram          (k = C chunk)
  per 128-token tile t:
    qk_ps [tok, 1024] = xT-chunks^T @ w_qkv[:, :1024]   (PE, 8 MM)
    q_bf, k_bf <- ACT cast; squares on ACT/DVE; fold+reduce -> |q|^2,|k|^2 per head
    s = 1/sqrt(max(|q|^2*|k|^2, eps^2)/scale^2)         (ACT sqrt + DVE recip)
    ks = k_bf * s (broadcast)                           (GPSIMD)
    attn_ps[:, pair] += q2^T @ ks2   2x2 head-pair packing, one PSUM bank,
                        start=False accumulate onto a memset bank (interleaved
                        chains must not use start_tensor_calc: it marks the
                        whole 2KB zero-region pending-zero)
  softmax per [64,64] head block (shift by max, ACT exp w/ accum sums)
  per token-chunk pair (512 tokens x 2):
    vT_bf[m] = w_v-chunk^T @ xT    (PE, stationary reused over chunk pair)
    outT_ps  = attn-block @ vT     (row-group packed odd/even head MMs)
    yT_ps    = w_proj-chunk^T @ outT  -> yT dram (host transposes back)
"""

import os
import sys

import numpy as np

for _p in ("/opt/trn_rl_repo", "/root/.axon_site/_ro/trn_rl_repo"):
    if _p not in sys.path and os.path.isdir(_p):
        sys.path.insert(0, _p)

os.environ.setdefault("MYCRO_LOCAL_CACHE", "1")

NUM_HEADS = 8
B, N, C = 8, 4096, 512
D = C // NUM_HEADS          # 64
SCALE = float(D) ** -0.5
EPS = 1e-12
P = 128                     # partitions / tile height
KC = C // P                 # 4 contraction chunks
NPAIR = NUM_HEADS // 2      # 4 head pairs


def _build(n_tokens=N, with_bias=False, debug_taps=False):
    """Build + compile the per-core Bass program. Returns nc."""
    import concourse.bacc as bacc
    import concourse.mybir as mybir
    import concourse.tile as tile
    from concourse.bass import ts

    f32 = mybir.dt.float32
    bf16 = mybir.dt.bfloat16
    AF = mybir.ActivationFunctionType
    AX = mybir.AxisListType

    n = n_tokens
    nt = n // P             # 128-token tiles
    tch = n // 512          # 512-token chunks
    assert tch % 2 == 0

    nc = bacc.Bacc("TRN2", target_bir_lowering=False, debug=False)

    xT = nc.dram_tensor("xT", [C, n], f32, kind="ExternalInput")
    w_qkv = nc.dram_tensor("w_qkv", [C, 3 * C], f32, kind="ExternalInput")
    w_proj = nc.dram_tensor("w_proj", [C, C], f32, kind="ExternalInput")
    if with_bias:
        b_qkv = nc.dram_tensor("b_qkv", [3 * C], f32, kind="ExternalInput")
        b_proj = nc.dram_tensor("b_proj", [C], f32, kind="ExternalInput")
    yT = nc.dram_tensor("yT", [C, n], f32, kind="ExternalOutput")

    taps = {}
    if debug_taps:
        for tname, shape in [
            ("tap_q0", [P, C]), ("tap_ks0", [P, C]), ("tap_s0", [P, NUM_HEADS]),
            ("tap_attn", [P, NPAIR * P]),
            ("tap_vT0", [P, 512]), ("tap_G0", [P, C]),
        ]:
            taps[tname] = nc.dram_tensor(tname, shape, f32, kind="ExternalOutput")

    with tile.TileContext(nc) as tc:
        with (
            tc.tile_pool(name="persist", bufs=1) as persist,
            # attn bank is allocated for the whole kernel so phase-2 psum
            # pools don't land on it and serialize behind the softmax reads
            tc.tile_pool(name="attnps", bufs=1, space="PSUM") as attn_pool,
        ):
            # ---------------- phase 0: loads (cast f32 -> bf16 in DMA) -------
            # Ordered so the first token tiles can start ASAP: the q,k halves
            # of w_qkv and the leading token-slices of xT come first; the v
            # columns and w_proj are only needed in phase 2.
            # chunk-merged tiles: one DMA covers all 4 C-chunks (k is a free
            # dim), cutting the SWDGE issue latency on the startup path
            xT_m = persist.tile([P, KC, n], bf16)
            w_m = persist.tile([P, KC, 3 * C], bf16)
            wp_m = persist.tile([P, KC, C], bf16)
            xT_sb = [xT_m[:, k, :] for k in range(KC)]
            w_bf = [w_m[:, k, :] for k in range(KC)]
            wp_bf = [wp_m[:, k, :] for k in range(KC)]
            xT_src = xT[:].rearrange("(k p) t -> p k t", p=P)
            w_src = w_qkv[:].rearrange("(k p) c -> p k c", p=P)
            # token pieces, finest first so tile 0 is runnable earliest
            pieces = []
            off = 0
            while off < n:
                sz = 256 if off < 512 and n > 512 else 512
                pieces.append((off, min(sz, n - off)))
                off += sz
            nc.gpsimd.dma_start(out=w_m[:, :, :C], in_=w_src[:, :, :C])
            for pc, (off, sz) in enumerate(pieces):
                nc.gpsimd.dma_start(
                    out=xT_m[:, :, off : off + sz], in_=xT_src[:, :, off : off + sz]
                )
                if pc == 0:
                    nc.gpsimd.dma_start(
                        out=w_m[:, :, C : 2 * C], in_=w_src[:, :, C : 2 * C]
                    )
                if pc == min(1, len(pieces) - 1):
                    nc.gpsimd.dma_start(out=w_m[:, :, 2 * C :], in_=w_src[:, :, 2 * C :])
                    nc.gpsimd.dma_start(
                        out=wp_m[:], in_=w_proj[:].rearrange("(k p) c -> p k c", p=P)
                    )

            if with_bias:
                bias_qk = persist.tile([P, 2 * C], f32)
                nc.gpsimd.dma_start(
                    out=bias_qk[:], in_=b_qkv[: 2 * C].partition_broadcast(P)
                )
                bias_v = persist.tile([P, KC], f32)
                nc.gpsimd.dma_start(
                    out=bias_v[:],
                    in_=b_qkv[2 * C :].rearrange("(m p) -> p m", p=P),
                )
                bias_p = persist.tile([P, KC], f32)
                nc.gpsimd.dma_start(
                    out=bias_p[:], in_=b_proj[:].rearrange("(m p) -> p m", p=P)
                )

            # ---------------- phase 1: qk + attn accumulation ---------------
            # Four accumulation chains share one PSUM bank: memset once and
            # accumulate with start=False (see module docstring).
            if True:
                attn_ps = attn_pool.tile([P, NPAIR * P], f32)
                nc.vector.memset(attn_ps[:], 0.0)

                vT_bf = [
                    persist.tile([P, n], bf16, name=f"vT_bf{m}") for m in range(KC)
                ]
                with (
                    tc.tile_pool(name="qkvps", bufs=2, space="PSUM") as qkvps,
                    tc.tile_pool(name="vtps", bufs=2, space="PSUM") as vtps,
                    tc.tile_pool(name="work", bufs=4) as work,
                    tc.tile_pool(name="smalls", bufs=4) as smalls,
                ):
                    # software-pipelined state (tile t emits C-matmuls of t-2)
                    st = {}

                    def emit_A(t):
                        qk_ps = qkvps.tile([P, 2 * C], f32)
                        for half in range(2):
                            for k in range(KC):
                                nc.tensor.matmul(
                                    qk_ps[:, ts(half, C)],
                                    xT_sb[k][:, ts(t, P)],
                                    w_bf[k][:, ts(half, C)],
                                    start=(k == 0),
                                    stop=(k == KC - 1),
                                )
                        q_bf = work.tile([P, C], bf16)
                        k_bf = work.tile([P, C], bf16)
                        sqq = work.tile([P, C], bf16)
                        sqk = work.tile([P, C], bf16)
                        if with_bias:
                            nc.vector.tensor_add(q_bf[:], qk_ps[:, :C], bias_qk[:, :C])
                            nc.vector.tensor_add(k_bf[:], qk_ps[:, C:], bias_qk[:, C:])
                            nc.scalar.activation(sqq[:], q_bf[:], AF.Square)
                            nc.vector.tensor_mul(sqk[:], k_bf[:], k_bf[:])
                        else:
                            nc.scalar.copy(q_bf[:], qk_ps[:, :C])
                            nc.scalar.copy(k_bf[:], qk_ps[:, C:])
                            nc.scalar.activation(sqq[:], q_bf[:], AF.Square)
                            nc.vector.tensor_mul(sqk[:], k_bf[:], k_bf[:])
                        # fold 64 -> 32 at 2x rate, then 1x reduce of 32
                        fq = work.tile([P, C // 2], bf16)
                        fk = work.tile([P, C // 2], bf16)
                        sqq3 = sqq[:].rearrange("p (h d) -> p h d", d=D)
                        sqk3 = sqk[:].rearrange("p (h d) -> p h d", d=D)
                        nc.vector.tensor_add(
                            fq[:].rearrange("p (h d) -> p h d", d=D // 2),
                            sqq3[:, :, 0 : D // 2],
                            sqq3[:, :, D // 2 : D],
                        )
                        nc.vector.tensor_add(
                            fk[:].rearrange("p (h d) -> p h d", d=D // 2),
                            sqk3[:, :, 0 : D // 2],
                            sqk3[:, :, D // 2 : D],
                        )
                        ssq = smalls.tile([P, NUM_HEADS], f32)
                        ssk = smalls.tile([P, NUM_HEADS], f32)
                        nc.vector.reduce_sum(
                            ssq[:], fq[:].rearrange("p (h d) -> p h d", d=D // 2), AX.X
                        )
                        nc.vector.reduce_sum(
                            ssk[:], fk[:].rearrange("p (h d) -> p h d", d=D // 2), AX.X
                        )
                        pp = smalls.tile([P, NUM_HEADS], f32)
                        nc.vector.tensor_mul(pp[:], ssq[:], ssk[:])
                        nc.vector.tensor_scalar_max(pp[:], pp[:], EPS * EPS)
                        st[t] = dict(q_bf=q_bf, k_bf=k_bf, pp=pp, qk_ps=qk_ps)

                    def emit_scale(t):
                        # sqrt on ACT (one tile late so its DVE deps are ready),
                        # then reciprocal and the broadcasted k*s on gpsimd
                        d = st[t]
                        rt = smalls.tile([P, NUM_HEADS], f32)
                        nc.scalar.activation(
                            rt[:], d["pp"][:], AF.Sqrt, scale=1.0 / (SCALE * SCALE)
                        )
                        s = smalls.tile([P, NUM_HEADS], f32)
                        nc.vector.reciprocal(s[:], rt[:])
                        ks_bf = work.tile([P, C], bf16)
                        nc.vector.tensor_mul(
                            ks_bf[:].rearrange("p (h d) -> p h d", d=D),
                            d["k_bf"][:].rearrange("p (h d) -> p h d", d=D),
                            s[:].to_broadcast((P, NUM_HEADS, D)),
                        )
                        d["ks_bf"] = ks_bf
                        d["s"] = s
                        if debug_taps and t == 0:
                            nc.gpsimd.dma_start(out=taps["tap_q0"][:], in_=d["q_bf"][:])
                            nc.gpsimd.dma_start(out=taps["tap_ks0"][:], in_=ks_bf[:])
                            nc.gpsimd.dma_start(out=taps["tap_s0"][:], in_=s[:])

                    def emit_C(t):
                        d = st.pop(t)
                        for pr in range(NPAIR):
                            nc.tensor.matmul(
                                attn_ps[:, ts(pr, P)],
                                d["q_bf"][:, ts(pr, P)],
                                d["ks_bf"][:, ts(pr, P)],
                                start=False,
                                stop=False,
                                skip_group_check=True,
                            )

                    # vT matmuls are independent of the attn pipeline and ride
                    # the same xT load frontier -- interleave them as PE
                    # gap-fillers, one chunk-pair group (stationary reused x2)
                    # every other tile; the DVE copy doubles as the bf16 cast.
                    def emit_B(gp):
                        m, tcp = gp % KC, gp // KC
                        vt_ps = [
                            vtps.tile([P, 512], f32, name="vt_ps") for _ in range(2)
                        ]
                        for k in range(KC):
                            for c2 in range(2):
                                nc.tensor.matmul(
                                    vt_ps[c2][:],
                                    w_bf[k][:, 2 * C + m * P : 2 * C + (m + 1) * P],
                                    xT_sb[k][:, ts(2 * tcp + c2, 512)],
                                    start=(k == 0),
                                    stop=(k == KC - 1),
                                )
                        for c2 in range(2):
                            tc_i = 2 * tcp + c2
                            if with_bias:
                                nc.scalar.add(
                                    vT_bf[m][:, ts(tc_i, 512)],
                                    vt_ps[c2][:],
                                    bias_v[:, m : m + 1],
                                )
                            else:
                                nc.vector.tensor_copy(
                                    vT_bf[m][:, ts(tc_i, 512)], vt_ps[c2][:]
                                )

                    DELAY = 2
                    BP_START = 8
                    n_bpairs = KC * tch // 2
                    next_gp = 0
                    for t in range(nt + DELAY):
                        if t < nt:
                            emit_A(t)
                        if 1 <= t <= nt:
                            emit_scale(t - 1)
                        if t >= DELAY:
                            emit_C(t - DELAY)
                        if (
                            t >= BP_START
                            and (t - BP_START) % 2 == 0
                            and next_gp < n_bpairs - 4
                        ):
                            emit_B(next_gp)
                            next_gp += 1
                    # trailing groups overlap the softmax chain on PE
                    for gp in range(next_gp, n_bpairs):
                        emit_B(gp)

                # ---------------- softmax on [64,64] head blocks -------------
                # head h: rows (h%2)*64..+64, cols (h//2)*128+(h%2)*64..+64
                # The 1/sum normalizer is NOT applied here -- it is folded into
                # the per-partition scale of the G = attn @ w_proj cast below.
                mx = persist.tile([P, NPAIR], f32)
                sums = persist.tile([P, NPAIR], f32)
                rs = persist.tile([P, NPAIR], f32)
                attn3 = attn_ps[:].rearrange("r (pr c) -> r pr c", c=P)
                nc.vector.reduce_max(mx[0:D, :], attn3[0:D, :, 0:D], AX.X, negate=True)
                nc.vector.reduce_max(mx[D:P, :], attn3[D:P, :, D:P], AX.X, negate=True)
                attn_eb = persist.tile([P, NPAIR * P], bf16)
                if debug_taps:
                    nc.vector.memset(attn_eb[:], 0.0)
                for h in range(NUM_HEADS):
                    pr, odd = h // 2, h % 2
                    r0 = odd * D
                    c0 = pr * P + odd * D
                    nc.scalar.activation(
                        attn_eb[r0 : r0 + D, c0 : c0 + D],
                        attn_ps[r0 : r0 + D, c0 : c0 + D],
                        AF.Exp,
                        bias=mx[r0 : r0 + D, pr : pr + 1],
                        accum_out=sums[r0 : r0 + D, pr : pr + 1],
                    )
                nc.vector.reciprocal(rs[:], sums[:])
                if debug_taps:
                    nc.gpsimd.dma_start(out=taps["tap_attn"][:], in_=attn_eb[:])

            # ------------- phase 2: G = softmax(attn) @ w_proj, then yT ------
            with (
                tc.tile_pool(name="tps", bufs=1, space="PSUM") as tpool,
                tc.tile_pool(name="gps", bufs=2, space="PSUM") as gps,
                tc.tile_pool(name="ytps", bufs=4, space="PSUM") as ytps,
                tc.tile_pool(name="ytsb", bufs=4) as ytsb,
            ):
                # G[(h,d), c'] = sum_e softmax-unnormalized attn_h[d,e] *
                # w_proj[h*64+e, c'], scaled by rs[(h,d)] at the PSUM cast.
                # Then yT = G^T-contraction over (h,d) with vT -- the outT
                # stage disappears entirely (y = v @ (attn @ w_proj)).
                ident = persist.tile([P, P], bf16)
                from concourse.masks import make_identity

                make_identity(nc, ident[:])
                attnT_bf = persist.tile([P, NPAIR * P], bf16)
                nc.vector.memset(attnT_bf[:], 0.0)
                G_bf = [persist.tile([P, C], bf16, name=f"G_bf{j}") for j in range(KC)]

                def emit_G():
                    tps_t = tpool.tile([P, NPAIR * P], bf16)
                    nc.vector.memset(tps_t[:].bitcast(mybir.dt.float32), 0.0)
                    for h in range(NUM_HEADS):
                        pr, odd = h // 2, h % 2
                        r0 = odd * D
                        c0 = pr * P + odd * D
                        nc.tensor.transpose(
                            tps_t[r0 : r0 + D, c0 : c0 + D],
                            attn_eb[r0 : r0 + D, c0 : c0 + D],
                            ident[r0 : r0 + D, r0 : r0 + D],
                        )
                    for j in range(KC):
                        nc.scalar.copy(attnT_bf[:, ts(j, P)], tps_t[:, ts(j, P)])
                    for j in range(KC):
                        g_ps = gps.tile([P, C], f32)
                        for odd in range(2):
                            r0 = odd * D
                            nc.tensor.matmul(
                                g_ps[r0 : r0 + D, :],
                                attnT_bf[r0 : r0 + D, j * P + r0 : j * P + r0 + D],
                                wp_bf[j][r0 : r0 + D, :],
                            )
                        # fold the softmax 1/sum (per output row (h,d)) here
                        nc.scalar.mul(G_bf[j][:], g_ps[:], rs[:, j : j + 1])

                emit_G()
                if debug_taps:
                    nc.gpsimd.dma_start(out=taps["tap_vT0"][:], in_=vT_bf[0][:, 0:512])
                    nc.gpsimd.dma_start(out=taps["tap_G0"][:], in_=G_bf[0][:])

                # yT = G^T-contraction with vT; stationary G reused x4 chunks
                ngrp = min(4, tch)
                for tcq in range(tch // ngrp):
                    for mp in range(KC):
                        yt_ps = [
                            ytps.tile([P, 512], f32, name="yt_ps") for _ in range(ngrp)
                        ]
                        for j in range(KC):
                            for cg in range(ngrp):
                                nc.tensor.matmul(
                                    yt_ps[cg][:],
                                    G_bf[j][:, ts(mp, P)],
                                    vT_bf[j][:, ts(ngrp * tcq + cg, 512)],
                                    start=(j == 0),
                                    stop=(j == KC - 1),
                                )
                        for cg in range(ngrp):
                            tc_i = ngrp * tcq + cg
                            yt_sb = ytsb.tile([P, 512], f32)
                            if with_bias:
                                nc.scalar.add(
                                    yt_sb[:], yt_ps[cg][:], bias_p[:, mp : mp + 1]
                                )
                            else:
                                nc.vector.tensor_copy(yt_sb[:], yt_ps[cg][:])
                            nc.sync.dma_start(
                                yT[ts(mp, P), ts(tc_i, 512)], yt_sb[:]
                            )

    nc.compile()
    return nc


_CACHE = {}


def _get_nc(n_tokens=N, with_bias=False):
    key = (n_tokens, with_bias)
    if key not in _CACHE:
        _CACHE[key] = _build(n_tokens, with_bias)
    return _CACHE[key]


def run(inputs, trace=False, n_tokens=N):
    """Run on 8 NeuronCores; returns (y [B,N,C] f32, BassKernelResults)."""
    from concourse import bass_utils

    x = np.asarray(inputs["x"], dtype=np.float32)
    w_qkv = np.asarray(inputs["w_qkv"], dtype=np.float32)
    b_qkv = np.asarray(inputs["b_qkv"], dtype=np.float32)
    w_proj = np.asarray(inputs["w_proj"], dtype=np.float32)
    b_proj = np.asarray(inputs["b_proj"], dtype=np.float32)

    with_bias = bool(np.any(b_qkv) or np.any(b_proj))
    nc = _get_nc(n_tokens, with_bias)

    in_maps = []
    for b in range(B):
        m = {
            "xT": np.ascontiguousarray(x[b].T),
            "w_qkv": w_qkv,
            "w_proj": w_proj,
        }
        if with_bias:
            m["b_qkv"] = b_qkv
            m["b_proj"] = b_proj
        in_maps.append(m)

    res = bass_utils.run_bass_kernel_spmd(
        nc, in_maps, core_ids=list(range(B)), trace=trace
    )
    out = np.stack(
        [np.ascontiguousarray(res.results[i]["yT"].T) for i in range(B)], axis=0
    )
    return out.astype(np.float32), res


def kernel(**inputs):
    out, _ = run(inputs)
    return out
